# revision 1
# baseline (speedup 1.0000x reference)
"""Involution2d Bass kernel for 8 trn2 NeuronCores.

Sharding: core = 2*b + half  (b = batch 0..3, half = group-half 0..1).
Each core computes out[b, half*128:(half+1)*128, :, :].

Math: ker = A @ x[b] + b_span  with A = w_span @ w_reduce folded on host
(rank-64 factorization folded; exact up to fp rounding).
out[c,p] = sum_kk ker[g(c),kk,p] * xpad[c, p+delta_kk]

Mapping:
 - ker-gen: PE matmuls (K=256 in 2 chunks), rows permuted kk-major (j = kk*8+g).
 - per tap kk: PE "replication" matmul (selection matrix) broadcasts the 8
   group-rows of tap kk to all 128 channel partitions (PSUM).
 - DVE tensor_tensor multiplies shifted xpad view by replicated ker (PSUM src).
 - PE identity matmuls accumulate the 49 tap products in PSUM.
"""
import numpy as np
from contextlib import ExitStack

B, C, H, W = 4, 256, 64, 64
G, K, PAD, R = 16, 7, 3, 4
HW = H * W
P = 128          # partitions / channels per core
NQ = 4           # pixel chunks
QPIX = HW // NQ  # 1024 pixels per quarter (16 image rows)
QROWS = H // NQ  # 16
JPAD = 512       # padded permuted-ker rows (392 -> 512)

_CACHE = {}


def _build_nc():
    import concourse.mybir as mybir
    import concourse.tile as tile
    from concourse import bacc

    f32 = mybir.dt.float32
    nc = bacc.Bacc("TRN2", target_bir_lowering=False, debug=False)

    xb = nc.dram_tensor("xb", (P, 2, H, W), f32, kind="ExternalInput")
    at = nc.dram_tensor("at", (P, 2, JPAD), f32, kind="ExternalInput")
    bias = nc.dram_tensor("bias", (P, 4), f32, kind="ExternalInput")
    bf16 = mybir.dt.bfloat16
    rep = nc.dram_tensor("rep", (P, 16, P), bf16, kind="ExternalInput")
    ident = nc.dram_tensor("ident", (P, P), mybir.dt.bfloat16, kind="ExternalInput")
    half_sel = nc.dram_tensor("half_sel", (P, 2), f32, kind="ExternalInput")
    out = nc.dram_tensor("out", (P, HW), f32, kind="ExternalOutput")

    with tile.TileContext(nc) as tc:
        with ExitStack() as ctx:
            const = ctx.enter_context(tc.tile_pool(name="const", bufs=1))
            ps_kg = ctx.enter_context(tc.tile_pool(name="ps_kg", bufs=1, space="PSUM"))
            ps_kerb = ctx.enter_context(tc.tile_pool(name="ps_kerb", bufs=2, space="PSUM"))
            ps_acc = ctx.enter_context(tc.tile_pool(name="ps_acc", bufs=1, space="PSUM"))
            sb_prod = ctx.enter_context(tc.tile_pool(name="sb_prod", bufs=4))
            sb_out = ctx.enter_context(tc.tile_pool(name="sb_out", bufs=2))

            x_sb = const.tile([P, 2, H, W], f32)
            at_sb = const.tile([P, 2, JPAD], f32)
            bias_sb = const.tile([P, 4], f32)
            rep_sb = const.tile([P, 16, P], bf16)
            id_sb = const.tile([P, P], bf16)
            hsel_sb = const.tile([P, 2], f32)
            ker_sb = const.tile([P, 4, HW], bf16)
            xpad7 = const.tile([P, K, H + 6, W], bf16)

            nc.sync.dma_start(x_sb[:], xb[:])
            nc.sync.dma_start(at_sb[:], at[:])
            nc.sync.dma_start(bias_sb[:], bias[:])
            nc.sync.dma_start(rep_sb[:], rep[:])
            nc.sync.dma_start(id_sb[:], ident[:])
            nc.sync.dma_start(hsel_sb[:], half_sel[:])

            # ---- xpad: zero border + our half's channels via PE select ----
            # x_half[c, :, :] = x_sb[:, half]; select via matmul with hsel?
            # Simpler: both halves' copies cost 2 ACT passes; select on host
            # instead: host sends xb with OUR half's 128 channels in slot 0.
            nc.vector.memset(xpad7[:], 0.0)
            for dj in range(K):
                s = dj - 3
                a, b = max(0, -s), min(W, W - s)
                nc.scalar.copy(
                    xpad7[:, dj, 3:3 + H, a:b],
                    x_sb[:, 0, :, a + s:b + s],
                )

            # ---- ker-gen: ker_sb[:, m, :] = (at[:, :, m-tile].T @ x) + bias ----
            for m in range(4):
                for n in range(8):
                    kg = ps_kg.tile([P, 512], f32)
                    for k in range(2):
                        nc.tensor.matmul(
                            kg[:],
                            at_sb[:, k, m * P:(m + 1) * P],
                            x_sb[:, k].rearrange("p h w -> p (h w)")[:, n * 512:(n + 1) * 512],
                            start=(k == 0), stop=(k == 1),
                        )
                    nc.scalar.add(
                        ker_sb[:, m, n * 512:(n + 1) * 512], kg[:],
                        bias_sb[:, m:m + 1],
                    )

            # ---- main loop: quarters x taps ----
            import concourse.mybir as _mb
            NT = K * K
            LOOKAHEAD = 2

            sb_kerb = ctx.enter_context(tc.tile_pool(name="sb_kerb", bufs=4))

            def emit_repl(q, kk):
                mt, tt = kk // 16, kk % 16
                kerb = ps_kerb.tile([P, QPIX], f32, tag="kerb")
                rg = 32 * ((tt % 16) // 4)
                for hh in range(2):
                    nc.tensor.matmul(
                        kerb[:, hh * 512:(hh + 1) * 512],
                        rep_sb[rg:rg + 32, tt, :],
                        ker_sb[rg:rg + 32, mt, q * QPIX + hh * 512:q * QPIX + (hh + 1) * 512],
                        start=True, stop=True,
                        tile_position=(rg, 0),
                    )
                kerbS = sb_kerb.tile([P, QPIX], bf16, tag="kerbS")
                nc.scalar.copy(kerbS[:], kerb[:])
                return kerbS

            for q in range(NQ):
                acc = ps_acc.tile([P, QPIX], f32)
                r0 = q * QROWS
                buckets = [[kk for kk in range(NT) if ((kk % 16) // 4) == r]
                           for r in range(4)]
                order = []
                while any(buckets):
                    for bkt in buckets:
                        if bkt:
                            order.append(bkt.pop(0))
                kerbs = {kk: emit_repl(q, kk) for kk in order[:LOOKAHEAD]}
                for i, kk in enumerate(order):
                    di, dj = kk // K, kk % K
                    prod = sb_prod.tile([P, QROWS, W], bf16)
                    nc.vector.tensor_tensor(
                        out=prod[:],
                        in0=xpad7[:, dj, di + r0: di + r0 + QROWS, :],
                        in1=kerbs.pop(kk)[:].rearrange("p (h w) -> p h w", w=W),
                        op=_mb.AluOpType.mult,
                    )
                    if i + LOOKAHEAD < NT:
                        nkk = order[i + LOOKAHEAD]
                        kerbs[nkk] = emit_repl(q, nkk)
                    pr = prod[:].rearrange("p h w -> p (h w)")
                    for hh in range(2):
                        nc.tensor.matmul(
                            acc[:, hh * 512:(hh + 1) * 512],
                            id_sb[:],
                            pr[:, hh * 512:(hh + 1) * 512],
                            start=(i == 0), stop=(i == NT - 1),
                        )
                o_sb = sb_out.tile([P, QPIX], f32)
                nc.scalar.copy(o_sb[:], acc[:])
                nc.sync.dma_start(out[:, q * QPIX:(q + 1) * QPIX], o_sb[:])

    nc.compile()
    return nc


def _host_inputs(x, w_reduce, w_span, b_span):
    A = (w_span.astype(np.float64) @ w_reduce.astype(np.float64)).astype(np.float32)
    import ml_dtypes as _md
    ident = np.eye(P, dtype=_md.bfloat16)
    rep = np.zeros((P, 16, P), dtype=np.float32)
    for p in range(P):
        for m in range(P):
            t = p // 8
            if p == t * 8 + m // 16:
                rep[p, t, m] = 1.0
    # rep[p, t, m] = 1 iff p == t*8 + m//16
    import ml_dtypes
    rep = np.zeros((P, 16, P), dtype=np.float32)
    for t in range(16):
        for m in range(P):
            rep[t * 8 + m // 16, t, m] = 1.0
    rep = rep.astype(ml_dtypes.bfloat16)

    in_maps = []
    for core in range(8):
        b, half = core // 2, core % 2
        # permuted fold: j = kk*8 + g  ->  A row (half*8+g)*49 + kk
        Ap = np.zeros((JPAD, C), dtype=np.float32)
        bp = np.zeros((JPAD,), dtype=np.float32)
        for kk in range(K * K):
            for g in range(8):
                j = kk * 8 + g
                src = (half * 8 + g) * (K * K) + kk
                Ap[j] = A[src]
                bp[j] = b_span[src]
        at = np.ascontiguousarray(
            Ap.T.reshape(2, P, JPAD).transpose(1, 0, 2))  # [P, 2, JPAD]
        bias = np.ascontiguousarray(bp.reshape(4, P).T)   # [P, 4]
        xh = x[b, half * P:(half + 1) * P]                # [128, H, W] our half
        xo = x[b, (1 - half) * P:(2 - half) * P]          # other half
        xb_arr = np.stack([xh, xo], axis=1)               # [P, 2, H, W]
        # ker-gen contracts over channel chunks k=0 (rows 0..127) and k=1:
        # chunk k must hold x channels k*128..k*128+127 in ORIGINAL order.
        # With xb[:,0]=our half, xb[:,1]=other: the A columns must be permuted
        # to match: columns [half*128:(half+1)*128] first, then the rest.
        colperm = np.concatenate([
            np.arange(half * P, (half + 1) * P),
            np.arange((1 - half) * P, (2 - half) * P)])
        Ap2 = Ap[:, colperm]
        at = np.ascontiguousarray(
            Ap2.T.reshape(2, P, JPAD).transpose(1, 0, 2))
        hsel = np.zeros((P, 2), dtype=np.float32)
        hsel[:, 0] = 1.0
        in_maps.append({
            "xb": np.ascontiguousarray(xb_arr, dtype=np.float32),
            "at": at.astype(np.float32),
            "bias": bias.astype(np.float32),
            "rep": rep,
            "ident": ident,
            "half_sel": hsel,
        })
    return in_maps


def kernel(x, w_reduce, w_span, b_span):
    from concourse import bass_utils
    x = np.asarray(x, dtype=np.float32)
    w_reduce = np.asarray(w_reduce, dtype=np.float32)
    w_span = np.asarray(w_span, dtype=np.float32)
    b_span = np.asarray(b_span, dtype=np.float32)

    if "nc" not in _CACHE:
        _CACHE["nc"] = _build_nc()
    nc = _CACHE["nc"]

    in_maps = _host_inputs(x, w_reduce, w_span, b_span)
    res = bass_utils.run_bass_kernel_spmd(nc, in_maps, core_ids=list(range(8)))

    out = np.empty((B, C, H, W), dtype=np.float32)
    for core in range(8):
        b, half = core // 2, core % 2
        out[b, half * P:(half + 1) * P] = res.results[core]["out"].reshape(P, H, W)
    return out



# revision 2
# speedup vs baseline: 1.2765x; 1.2765x over previous
"""Involution2d Bass kernel for 8 trn2 NeuronCores.

Sharding: core = 2*b + half  (b = batch 0..3, half = group-half 0..1).
Each core computes out[b, half*128:(half+1)*128, :, :].

Math: ker = A @ x[b] + b_span with A = w_span @ w_reduce folded on host.
out[c,p] = sum_kk ker[g(c),kk,p] * xpad[c, p+delta_kk]

Dataflow per core:
 - ker-gen: bf16 PE matmuls (K=256 in 2 chunks) -> PSUM -> ACT copy+bias
   into ker_sb bf16. Row layout per m-tile: r = g*16 + tt (g-major), where
   tap kk = mt*16 + tt.
 - kerb delivery (broadcast each g-row to its 16 channels):
   * DMA path: per (quarter, m-tile), 16 strided SBUF->SBUF DMAs
     (dst[c16::16] <- ker_sb rows) fill kerbS [128, 16, QPIX] bf16.
   * PE path: selection matmul (K=128) -> PSUM -> ACT copy to bf16.
 - DVE tensor_tensor (bf16 2x mode) multiplies shifted xpad view by kerb.
 - PE identity matmuls accumulate the 49 tap products in PSUM per quarter.
"""
import numpy as np
from contextlib import ExitStack

B, C, H, W = 4, 256, 64, 64
G, K, PAD, R = 16, 7, 3, 4
HW = H * W
P = 128          # partitions / channels per core
NQ = 4           # pixel chunks (quarters; 16 image rows each)
QPIX = HW // NQ  # 1024
QROWS = H // NQ  # 16
NMT = 4          # m-tiles of 16 tap slots (64 slots >= 49 taps)
HP = H + 2 * PAD + 0  # 70 padded rows
WP = W + 2 * PAD      # 70 padded cols

# (q, mt) combos whose kerb goes via the 16-DMA strided replication path;
# the rest use PE selection matmul + ACT copy.
DMA_MTS = {(q, mt) for q in range(NQ) for mt in (0, 1)} | {(0, 2)}

_CACHE = {}


def _build_nc():
    import concourse.mybir as mybir
    import concourse.tile as tile
    from concourse import bacc

    f32 = mybir.dt.float32
    bf16 = mybir.dt.bfloat16
    nc = bacc.Bacc("TRN2", target_bir_lowering=False, debug=False)

    xb = nc.dram_tensor("xb", (P, 2, HW), bf16, kind="ExternalInput")
    at = nc.dram_tensor("at", (P, 2, NMT, P), bf16, kind="ExternalInput")
    bias = nc.dram_tensor("bias", (P, NMT), f32, kind="ExternalInput")
    sel = nc.dram_tensor("sel", (P, 16, P), bf16, kind="ExternalInput")
    ident = nc.dram_tensor("ident", (P, P), bf16, kind="ExternalInput")
    out = nc.dram_tensor("out", (P, HW), f32, kind="ExternalOutput")

    with tile.TileContext(nc) as tc:
        with ExitStack() as ctx:
            const = ctx.enter_context(tc.tile_pool(name="const", bufs=1))
            ps_kg = ctx.enter_context(tc.tile_pool(name="ps_kg", bufs=2, space="PSUM"))
            ps_kb = ctx.enter_context(tc.tile_pool(name="ps_kb", bufs=2, space="PSUM"))
            ps_acc = ctx.enter_context(tc.tile_pool(name="ps_acc", bufs=2, space="PSUM"))
            sb_kb = ctx.enter_context(tc.tile_pool(name="sb_kb", bufs=2))
            sb_kb1 = ctx.enter_context(tc.tile_pool(name="sb_kb1", bufs=4))
            sb_prod = ctx.enter_context(tc.tile_pool(name="sb_prod", bufs=4))
            sb_out = ctx.enter_context(tc.tile_pool(name="sb_out", bufs=2))

            x_sb = const.tile([P, 2, HW], bf16)
            at_sb = const.tile([P, 2, NMT, P], bf16)
            bias_sb = const.tile([P, NMT], f32)
            sel_sb = const.tile([P, 16, P], bf16)
            id_sb = const.tile([P, P], bf16)
            ker_sb = const.tile([P, NMT, HW], bf16)
            xpad = const.tile([P, HP, WP], bf16)

            nc.sync.dma_start(x_sb[:], xb[:])
            nc.sync.dma_start(at_sb[:], at[:])
            nc.sync.dma_start(bias_sb[:], bias[:])
            nc.sync.dma_start(sel_sb[:], sel[:])
            nc.sync.dma_start(id_sb[:], ident[:])

            # xpad: zero border via gpsimd memset (idle engine), interior via
            # DVE 4x bf16 copy (strided rows of 70, contiguous 64 cols).
            nc.gpsimd.memset(xpad[:], 0.0)
            nc.vector.tensor_copy(
                xpad[:, PAD:PAD + H, PAD:PAD + W],
                x_sb[:, 0].rearrange("p (h w) -> p h w", w=W),
            )

            # ---- ker-gen: ker_sb[:, mt, :] = (at[:, :, mt].T @ x) + bias ----
            for mt in range(NMT):
                for n in range(8):
                    kg = ps_kg.tile([P, 512], f32)
                    for k in range(2):
                        nc.tensor.matmul(
                            kg[:],
                            at_sb[:, k, mt, :],
                            x_sb[:, k, n * 512:(n + 1) * 512],
                            start=(k == 0), stop=(k == 1),
                        )
                    nc.scalar.add(
                        ker_sb[:, mt, n * 512:(n + 1) * 512], kg[:],
                        bias_sb[:, mt:mt + 1],
                    )

            # ---- main loop ----
            import concourse.mybir as _mb

            def xview(q, kk, h0=0, nr=QROWS):
                di, dj = kk // K, kk % K
                r0 = q * QROWS + di + h0
                return xpad[:, r0:r0 + nr, dj:dj + W]

            for q in range(NQ):
                acc = ps_acc.tile([P, QPIX], f32)
                qs = q * QPIX
                nacc = [0, 0]
                # count accumulation matmuls per half for start/stop flags
                tot = [0, 0]
                for mt in range(NMT):
                    ntap = min(49 - mt * 16, 16)
                    tot[0] += ntap
                    tot[1] += ntap

                for mt in range(NMT):
                    ntap = min(49 - mt * 16, 16)
                    if (q, mt) in DMA_MTS:
                        kb = sb_kb.tile([P, 16, QPIX], bf16)
                        src = ker_sb[:, mt, qs:qs + QPIX]
                        for c16 in range(16):
                            nc.sync.dma_start(kb[:][c16::16], src)
                        for tt in range(ntap):
                            kk = mt * 16 + tt
                            prod = sb_prod.tile([P, QROWS, W], bf16)
                            nc.vector.tensor_tensor(
                                out=prod[:],
                                in0=xview(q, kk),
                                in1=kb[:, tt].rearrange("p (h w) -> p h w", w=W),
                                op=_mb.AluOpType.mult,
                            )
                            pr = prod[:].rearrange("p h w -> p (h w)")
                            for h in range(2):
                                nc.tensor.matmul(
                                    acc[:, h * 512:(h + 1) * 512],
                                    id_sb[:],
                                    pr[:, h * 512:(h + 1) * 512],
                                    start=(nacc[h] == 0),
                                    stop=(nacc[h] == tot[h] - 1),
                                )
                                nacc[h] += 1
                    else:
                        for tt in range(ntap):
                            kk = mt * 16 + tt
                            for h in range(2):
                                kps = ps_kb.tile([P, 512], f32)
                                nc.tensor.matmul(
                                    kps[:],
                                    sel_sb[:, tt, :],
                                    ker_sb[:, mt, qs + h * 512:qs + (h + 1) * 512],
                                    start=True, stop=True,
                                )
                                kbs = sb_kb1.tile([P, 512], bf16)
                                nc.scalar.copy(kbs[:], kps[:])
                                prod = sb_prod.tile([P, 8, W], bf16)
                                nc.vector.tensor_tensor(
                                    out=prod[:],
                                    in0=xview(q, kk, h0=8 * h, nr=8),
                                    in1=kbs[:].rearrange("p (h w) -> p h w", w=W),
                                    op=_mb.AluOpType.mult,
                                )
                                nc.tensor.matmul(
                                    acc[:, h * 512:(h + 1) * 512],
                                    id_sb[:],
                                    prod[:].rearrange("p h w -> p (h w)"),
                                    start=(nacc[h] == 0),
                                    stop=(nacc[h] == tot[h] - 1),
                                )
                                nacc[h] += 1

                o_sb = sb_out.tile([P, QPIX], f32)
                nc.scalar.copy(o_sb[:], acc[:])
                nc.sync.dma_start(out[:, qs:qs + QPIX], o_sb[:])

    nc.compile()
    return nc


def _host_inputs(x, w_reduce, w_span, b_span):
    import ml_dtypes
    bf = ml_dtypes.bfloat16
    A = (w_span.astype(np.float64) @ w_reduce.astype(np.float64)).astype(np.float32)

    ident = np.eye(P, dtype=bf)
    # sel[r=(g*16+tt), tt, c] = 1 iff r == (c//16)*16 + tt
    sel = np.zeros((P, 16, P), dtype=np.float32)
    for tt in range(16):
        for c in range(P):
            sel[(c // 16) * 16 + tt, tt, c] = 1.0
    sel = sel.astype(bf)

    in_maps = []
    for core in range(8):
        b, half = core // 2, core % 2
        # row layout: m-tile mt, row r = g*16 + tt -> A row (half*8+g)*49 + kk
        # with kk = mt*16 + tt (rows with kk >= 49 are zero-padded)
        Ap = np.zeros((NMT, P, C), dtype=np.float32)
        bp = np.zeros((NMT, P), dtype=np.float32)
        for mt in range(NMT):
            for tt in range(16):
                kk = mt * 16 + tt
                if kk >= K * K:
                    continue
                for g in range(8):
                    r = g * 16 + tt
                    src = (half * 8 + g) * (K * K) + kk
                    Ap[mt, r] = A[src]
                    bp[mt, r] = b_span[src]
        # contraction chunk k holds x channels: chunk 0 = our half, 1 = other
        colperm = np.concatenate([
            np.arange(half * P, (half + 1) * P),
            np.arange((1 - half) * P, (2 - half) * P)])
        Ap = Ap[:, :, colperm]
        # at[cin, k, mt, r] = Ap[mt, r, k*128 + cin]
        at = np.ascontiguousarray(Ap.transpose(2, 0, 1).reshape(2, P, NMT, P)
                                  .transpose(1, 0, 2, 3))
        bias = np.ascontiguousarray(bp.T)  # [P, NMT]

        xh = x[b, half * P:(half + 1) * P].reshape(P, HW)
        xo = x[b, (1 - half) * P:(2 - half) * P].reshape(P, HW)
        xb_arr = np.stack([xh, xo], axis=1)  # [P, 2, HW]
        in_maps.append({
            "xb": xb_arr.astype(bf),
            "at": at.astype(bf),
            "bias": bias.astype(np.float32),
            "sel": sel,
            "ident": ident,
        })
    return in_maps


def kernel(x, w_reduce, w_span, b_span):
    from concourse import bass_utils
    x = np.asarray(x, dtype=np.float32)
    w_reduce = np.asarray(w_reduce, dtype=np.float32)
    w_span = np.asarray(w_span, dtype=np.float32)
    b_span = np.asarray(b_span, dtype=np.float32)

    if "nc" not in _CACHE:
        _CACHE["nc"] = _build_nc()
    nc = _CACHE["nc"]

    in_maps = _host_inputs(x, w_reduce, w_span, b_span)
    res = bass_utils.run_bass_kernel_spmd(nc, in_maps, core_ids=list(range(8)))

    out = np.empty((B, C, H, W), dtype=np.float32)
    for core in range(8):
        b, half = core // 2, core % 2
        out[b, half * P:(half + 1) * P] = res.results[core]["out"].reshape(P, H, W)
    return out


# revision 4
# speedup vs baseline: 1.4204x; 1.1128x over previous
"""Involution2d Bass kernel for 8 trn2 NeuronCores.

Sharding: core = 2*b + half  (b = batch 0..3, half = group-half 0..1).
Each core computes out[b, half*128:(half+1)*128, :, :].

Math: ker = A @ x[b] + b_span with A = w_span @ w_reduce folded on host.
out[c,p] = sum_kk ker[g(c),kk,p] * xpad[c, p+delta_kk]

Dataflow per core:
 - ker-gen: bf16 PE matmuls (K=256 in 2 chunks) -> PSUM -> ACT copy+bias
   into ker_sb bf16. Row layout per m-tile: r = g*16 + tt (g-major), where
   tap kk = mt*16 + tt.
 - kerb delivery (broadcast each g-row to its 16 channels):
   * DMA path: per (quarter, m-tile), 16 strided SBUF->SBUF DMAs
     (dst[c16::16] <- ker_sb rows) fill kerbS [128, 16, QPIX] bf16.
   * PE path: selection matmul (K=128) -> PSUM -> ACT copy to bf16.
 - DVE tensor_tensor (bf16 2x mode) multiplies shifted xpad view by kerb.
 - PE identity matmuls accumulate the 49 tap products in PSUM per quarter.
"""
import numpy as np
from contextlib import ExitStack

B, C, H, W = 4, 256, 64, 64
G, K, PAD, R = 16, 7, 3, 4
HW = H * W
P = 128          # partitions / channels per core
NQ = 4           # pixel chunks (quarters; 16 image rows each)
QPIX = HW // NQ  # 1024
QROWS = H // NQ  # 16
NMT = 4          # m-tiles of 16 tap slots (64 slots >= 49 taps)
HP = H + 2 * PAD + 0  # 70 padded rows
WP = W + 2 * PAD      # 70 padded cols

# (q, mt) combos whose kerb goes via the 16-DMA strided replication path;
# the rest use PE selection matmul + ACT copy.
DMA_MTS = {(q, mt) for q in range(NQ) for mt in (0, 1)} | {(0, 2)}

_CACHE = {}


def _build_nc():
    import concourse.mybir as mybir
    import concourse.tile as tile
    from concourse import bacc

    f32 = mybir.dt.float32
    bf16 = mybir.dt.bfloat16
    nc = bacc.Bacc("TRN2", target_bir_lowering=False, debug=False)

    xb = nc.dram_tensor("xb", (P, 2, HW), bf16, kind="ExternalInput")
    at = nc.dram_tensor("at", (P, 2, NMT, P), bf16, kind="ExternalInput")
    bias = nc.dram_tensor("bias", (P, NMT), f32, kind="ExternalInput")
    sel = nc.dram_tensor("sel", (P, 16, P), bf16, kind="ExternalInput")
    ident = nc.dram_tensor("ident", (P, P), bf16, kind="ExternalInput")
    out = nc.dram_tensor("out", (P, HW), f32, kind="ExternalOutput")

    with tile.TileContext(nc) as tc:
        with ExitStack() as ctx:
            const = ctx.enter_context(tc.tile_pool(name="const", bufs=1))
            ps_kg = ctx.enter_context(tc.tile_pool(name="ps_kg", bufs=2, space="PSUM"))
            ps_kb = ctx.enter_context(tc.tile_pool(name="ps_kb", bufs=2, space="PSUM"))
            ps_acc = ctx.enter_context(tc.tile_pool(name="ps_acc", bufs=1, space="PSUM"))
            sb_kb = ctx.enter_context(tc.tile_pool(name="sb_kb", bufs=2))
            sb_kb1 = ctx.enter_context(tc.tile_pool(name="sb_kb1", bufs=4))
            sb_prod = ctx.enter_context(tc.tile_pool(name="sb_prod", bufs=4))
            sb_out = ctx.enter_context(tc.tile_pool(name="sb_out", bufs=2))

            x_sb = const.tile([P, 2, HW], bf16)
            at_sb = const.tile([P, 2, NMT, P], bf16)
            bias_sb = const.tile([P, NMT], f32)
            sel_sb = const.tile([P, 16, P], bf16)
            id_sb = const.tile([P, P], bf16)
            ker_sb = const.tile([P, NMT, HW], bf16)
            xpad = const.tile([P, HP, WP], bf16)

            nc.sync.dma_start(x_sb[:], xb[:])
            nc.sync.dma_start(at_sb[:], at[:])
            nc.sync.dma_start(bias_sb[:], bias[:])
            nc.sync.dma_start(sel_sb[:], sel[:])
            nc.sync.dma_start(id_sb[:], ident[:])

            # xpad: zero border via gpsimd memset (idle engine), interior via
            # DVE 4x bf16 copy (strided rows of 70, contiguous 64 cols).
            nc.gpsimd.memset(xpad[:], 0.0)
            nc.vector.tensor_copy(
                xpad[:, PAD:PAD + H, PAD:PAD + W],
                x_sb[:, 0].rearrange("p (h w) -> p h w", w=W),
            )

            # ---- ker-gen: ker_sb[:, mt, :] = (at[:, :, mt].T @ x) + bias ----
            for mt in range(NMT):
                for n in range(8):
                    kg = ps_kg.tile([P, 512], f32)
                    for k in range(2):
                        nc.tensor.matmul(
                            kg[:],
                            at_sb[:, k, mt, :],
                            x_sb[:, k, n * 512:(n + 1) * 512],
                            start=(k == 0), stop=(k == 1),
                        )
                    nc.scalar.add(
                        ker_sb[:, mt, n * 512:(n + 1) * 512], kg[:],
                        bias_sb[:, mt:mt + 1],
                    )

            # ---- main loop ----
            import concourse.mybir as _mb

            def xview(q, kk, h0=0, nr=QROWS):
                di, dj = kk // K, kk % K
                r0 = q * QROWS + di + h0
                return xpad[:, r0:r0 + nr, dj:dj + W]

            for q in range(NQ):
                acc = ps_acc.tile([P, QPIX], f32)
                qs = q * QPIX

                # Build the tap schedule for this quarter: list of
                # (kk, src_kind, src) where src_kind is "kb" (DMA-filled
                # batch tile + slot) or "pe" (needs REP matmul + ACT copy).
                taps = []
                for mt in range(NMT):
                    ntap = min(49 - mt * 16, 16)
                    if (q, mt) in DMA_MTS:
                        kb = sb_kb.tile([P, 16, QPIX], bf16)
                        src = ker_sb[:, mt, qs:qs + QPIX]
                        for c16 in range(16):
                            eng = nc.sync if c16 % 2 == 0 else nc.gpsimd
                            eng.dma_start(kb[:][c16::16], src)
                        for tt in range(ntap):
                            taps.append((mt * 16 + tt, "kb", (kb, tt)))
                    else:
                        for tt in range(ntap):
                            taps.append((mt * 16 + tt, "pe", (mt, tt)))
                ntot = len(taps)

                # Software-pipelined emission: REP(i+3), copy(i+2), TT(i+1),
                # ACC(i).  Stage state held in dicts keyed by tap index.
                reps = {}   # i -> kps psum tile (PE path only)
                kbs_ = {}   # i -> SBUF bf16 [P, QPIX] kerb for tap i
                prods = {}  # i -> prod tile

                def st_rep(i):
                    kk, kind, s = taps[i]
                    if kind != "pe":
                        return
                    mt, tt = s
                    kps = ps_kb.tile([P, QPIX], f32)
                    for h in range(2):
                        nc.tensor.matmul(
                            kps[:, h * 512:(h + 1) * 512],
                            sel_sb[:, tt, :],
                            ker_sb[:, mt, qs + h * 512:qs + (h + 1) * 512],
                            start=True, stop=True,
                        )
                    reps[i] = kps

                def st_copy(i):
                    kk, kind, s = taps[i]
                    if kind != "pe":
                        return
                    kbs = sb_kb1.tile([P, QPIX], bf16)
                    nc.scalar.copy(kbs[:], reps.pop(i)[:])
                    kbs_[i] = kbs

                def st_tt(i):
                    kk, kind, s = taps[i]
                    if kind == "kb":
                        kb, tt = s
                        in1 = kb[:, tt].rearrange("p (h w) -> p h w", w=W)
                    else:
                        in1 = kbs_.pop(i)[:].rearrange("p (h w) -> p h w", w=W)
                    prod = sb_prod.tile([P, QROWS, W], bf16)
                    nc.vector.tensor_tensor(
                        out=prod[:], in0=xview(q, kk), in1=in1,
                        op=_mb.AluOpType.mult,
                    )
                    prods[i] = prod

                def st_acc(i):
                    pr = prods.pop(i)[:].rearrange("p h w -> p (h w)")
                    for h in range(2):
                        nc.tensor.matmul(
                            acc[:, h * 512:(h + 1) * 512],
                            id_sb[:],
                            pr[:, h * 512:(h + 1) * 512],
                            start=(i == 0), stop=(i == ntot - 1),
                        )

                for i in range(ntot + 3):
                    if i < ntot:
                        st_rep(i)
                    if i - 1 >= 0 and i - 1 < ntot:
                        st_copy(i - 1)
                    if i - 2 >= 0 and i - 2 < ntot:
                        st_tt(i - 2)
                    if i - 3 >= 0:
                        st_acc(i - 3)

                o_sb = sb_out.tile([P, QPIX], f32)
                nc.scalar.copy(o_sb[:], acc[:])
                nc.sync.dma_start(out[:, qs:qs + QPIX], o_sb[:])

    nc.compile()
    return nc


def _host_inputs(x, w_reduce, w_span, b_span):
    import ml_dtypes
    bf = ml_dtypes.bfloat16
    A = (w_span.astype(np.float64) @ w_reduce.astype(np.float64)).astype(np.float32)

    ident = np.eye(P, dtype=bf)
    # sel[r=(g*16+tt), tt, c] = 1 iff r == (c//16)*16 + tt
    sel = np.zeros((P, 16, P), dtype=np.float32)
    for tt in range(16):
        for c in range(P):
            sel[(c // 16) * 16 + tt, tt, c] = 1.0
    sel = sel.astype(bf)

    in_maps = []
    for core in range(8):
        b, half = core // 2, core % 2
        # row layout: m-tile mt, row r = g*16 + tt -> A row (half*8+g)*49 + kk
        # with kk = mt*16 + tt (rows with kk >= 49 are zero-padded)
        Ap = np.zeros((NMT, P, C), dtype=np.float32)
        bp = np.zeros((NMT, P), dtype=np.float32)
        for mt in range(NMT):
            for tt in range(16):
                kk = mt * 16 + tt
                if kk >= K * K:
                    continue
                for g in range(8):
                    r = g * 16 + tt
                    src = (half * 8 + g) * (K * K) + kk
                    Ap[mt, r] = A[src]
                    bp[mt, r] = b_span[src]
        # contraction chunk k holds x channels: chunk 0 = our half, 1 = other
        colperm = np.concatenate([
            np.arange(half * P, (half + 1) * P),
            np.arange((1 - half) * P, (2 - half) * P)])
        Ap = Ap[:, :, colperm]
        # at[cin, k, mt, r] = Ap[mt, r, k*128 + cin]
        at = np.ascontiguousarray(Ap.transpose(2, 0, 1).reshape(2, P, NMT, P)
                                  .transpose(1, 0, 2, 3))
        bias = np.ascontiguousarray(bp.T)  # [P, NMT]

        xh = x[b, half * P:(half + 1) * P].reshape(P, HW)
        xo = x[b, (1 - half) * P:(2 - half) * P].reshape(P, HW)
        xb_arr = np.stack([xh, xo], axis=1)  # [P, 2, HW]
        in_maps.append({
            "xb": xb_arr.astype(bf),
            "at": at.astype(bf),
            "bias": bias.astype(np.float32),
            "sel": sel,
            "ident": ident,
        })
    return in_maps


def kernel(x, w_reduce, w_span, b_span):
    from concourse import bass_utils
    x = np.asarray(x, dtype=np.float32)
    w_reduce = np.asarray(w_reduce, dtype=np.float32)
    w_span = np.asarray(w_span, dtype=np.float32)
    b_span = np.asarray(b_span, dtype=np.float32)

    if "nc" not in _CACHE:
        _CACHE["nc"] = _build_nc()
    nc = _CACHE["nc"]

    in_maps = _host_inputs(x, w_reduce, w_span, b_span)
    res = bass_utils.run_bass_kernel_spmd(nc, in_maps, core_ids=list(range(8)))

    out = np.empty((B, C, H, W), dtype=np.float32)
    for core in range(8):
        b, half = core // 2, core % 2
        out[b, half * P:(half + 1) * P] = res.results[core]["out"].reshape(P, H, W)
    return out
